# revision 11
# baseline (speedup 1.0000x reference)
"""CAGAT MinSum layer (segment-softmax GNN message passing) on 8 TRN2 NeuronCores.

Strategy
--------
The per-edge feature pipeline collapses algebraically: node features are
scalars, so `att_input @ W_att.T` reduces to per-head scalar coefficients
    raw[e,k] = a_k*f_src[e] + b_k*f_dst[e] + c_k*m[e] + d_k
and the segment softmax + head-mean + scatter fuses into two segment sums
    u[n,k] = sum_{e->n} z[e,k],    t[n,k] = sum_{e->n} f_src[e]*z[e,k]
    out[n] = (scaler/8) * sum_k t[n,k] / (u[n,k] + eps)
with z[e,k] = exp(lrelu(raw) + p_k*m).  raw is bounded (|raw| < ~20), so the
max-subtraction in the reference softmax is unnecessary in f32, and since the
reference's seg_sum >= ~1e-3 its 1e-16 epsilon is negligible.

Sharding: nodes (and their incoming edges) are partitioned across the 8 cores
by destination, so each core owns its output slice and no collective is
needed.  On each core the edges are laid out host-side in a padded-CSR
"node-row" layout: partition p, block b of the SBUF plane holds the edges of
one node in a run of W_b columns (W_b = per-block padded max degree, blocks
degree-sorted).  The dst-side segment sums then become dense row reductions
(tensor_reduce over uniform-width column groups) - no device scatter.  Pad
slots are killed by setting their mask to a large value M with penalty<0 so
z underflows to exactly 0 (host-verified; falls back to an explicit validity
plane if the penalty is not sufficiently negative).
"""

import sys

sys.path.insert(0, "/opt/trn_rl_repo")

import numpy as np

N_NODES = 50000
N_EDGES = 1600000
HEADS = 8
N_CORES = 8
P = 128
EPS_DEN = 1e-12
M_BIG = 1000.0


# ---------------------------------------------------------------- host prep


def _fold_weights(W_proj, b_proj, W_att, b_att, cycle_penalty, min_sum_scaler):
    H = W_proj.shape[0]
    w = W_proj[:, 0].astype(np.float64)
    Wa = W_att.astype(np.float64)
    a = Wa[:, :H] @ w
    b = Wa[:, H : 2 * H] @ w
    c = Wa[:, 2 * H]
    d = (Wa[:, :H] + Wa[:, H : 2 * H]) @ b_proj.astype(np.float64) + b_att.astype(
        np.float64
    )
    p = cycle_penalty.astype(np.float64)
    s8 = float(min_sum_scaler[0]) / HEADS
    return (
        a.astype(np.float32),
        b.astype(np.float32),
        c.astype(np.float32),
        d.astype(np.float32),
        p.astype(np.float32),
        np.float32(s8),
    )


def _build_layout(dst):
    """Node->(core, partition, block) assignment + unified block widths."""
    n = N_NODES
    deg = np.bincount(dst, minlength=n)
    order = np.argsort(-deg, kind="stable")  # node ids in degree-desc order
    # rank r -> core r%8, j=r//8 -> block j//128, partition j%128
    npc = (n + N_CORES - 1) // N_CORES  # nodes per core (6250)
    nb = (npc + P - 1) // P  # blocks per core
    # node_of[c, j] = node id
    pad_n = npc * N_CORES
    nodes_pad = np.full(pad_n, -1, dtype=np.int64)
    nodes_pad[: len(order)] = order
    node_of = nodes_pad.reshape(npc, N_CORES).T  # [8, npc]

    # per-block width: max degree of any node in block i across all cores
    deg_of = np.where(node_of >= 0, deg[np.clip(node_of, 0, n - 1)], 0)  # [8, npc]
    pad_npc = nb * P
    deg_pad = np.zeros((N_CORES, pad_npc), dtype=np.int64)
    deg_pad[:, :npc] = deg_of
    blk_max = deg_pad.reshape(N_CORES, nb, P).max(axis=(0, 2))  # [nb]
    W = np.maximum(4, ((blk_max + 3) // 4) * 4).astype(np.int64)  # [nb]
    colbase = np.zeros(nb + 1, dtype=np.int64)
    colbase[1:] = np.cumsum(W)
    F = int(colbase[-1])

    # groups of consecutive blocks with equal width
    groups = []  # (block_start, count, width, col_offset)
    i = 0
    while i < nb:
        jx = i
        while jx < nb and W[jx] == W[i]:
            jx += 1
        groups.append((i, jx - i, int(W[i]), int(colbase[i])))
        i = jx
    return deg, order, node_of, nb, W, colbase, F, groups


def _build_planes(node_features, cycle_mask, src, dst, layout, use_valid):
    deg, order, node_of, nb, W, colbase, F, groups = layout
    n = N_NODES
    nf = node_features.astype(np.float32)

    # per-node placement
    rank = np.empty(n, dtype=np.int64)
    rank[order] = np.arange(n)
    core_of_node = rank % N_CORES
    j_of_node = rank // N_CORES
    part_of_node = j_of_node % P
    block_of_node = j_of_node // P

    # order edges by (core, j) of dst, then stable position within the node
    key = core_of_node[dst] * (node_of.shape[1] + 1) + j_of_node[dst]
    eorder = np.argsort(key, kind="stable")
    dsts = dst[eorder]
    srcs = src[eorder]
    msks = cycle_mask[eorder]
    # position of each edge within its node's run
    dd = deg[dsts]
    # cumulative position: since sorted by node, use counting
    first = np.zeros(len(dsts), dtype=bool)
    first[0] = True
    first[1:] = dsts[1:] != dsts[:-1]
    run_start = np.where(first, np.arange(len(dsts)), 0)
    run_start = np.maximum.accumulate(run_start)
    pos = np.arange(len(dsts)) - run_start
    del dd

    ce = core_of_node[dsts]
    pe = part_of_node[dsts]
    cole = colbase[block_of_node[dsts]] + pos
    flat = (ce * P + pe) * F + cole

    fs = np.zeros(N_CORES * P * F, dtype=np.float32)
    fd = np.zeros((N_CORES, P, F), dtype=np.float32)
    if use_valid:
        ms = np.zeros(N_CORES * P * F, dtype=np.float32)
        valid = np.zeros(N_CORES * P * F, dtype=np.float32)
        valid[flat] = 1.0
        valid = valid.reshape(N_CORES, P, F)
    else:
        ms = np.full(N_CORES * P * F, M_BIG, dtype=np.float32)
        valid = None
    fs[flat] = nf[srcs]
    ms[flat] = msks
    fs = fs.reshape(N_CORES, P, F)
    ms = ms.reshape(N_CORES, P, F)

    # fd plane: per (core, partition, block) = own-node feature, expanded
    nf_blk = np.zeros((N_CORES, P, nb), dtype=np.float32)  # own-node feature
    jj = j_of_node
    nf_blk[core_of_node, jj % P, jj // P] = nf
    for (b0, cnt, Wg, off) in groups:
        seg = nf_blk[:, :, b0 : b0 + cnt]  # [8, P, cnt]
        fd[:, :, off : off + cnt * Wg] = np.repeat(seg, Wg, axis=2)

    return fs, fd, ms, valid


def _check_pad_trick(coef, node_features):
    """exp(lrelu(c_k*M + b_k*f + d_k) + p_k*M) must underflow to 0 in f32."""
    a, b, c, d, p, s8 = coef
    f = node_features.astype(np.float64)
    worst = -np.inf
    for k in range(HEADS):
        t = c[k] * M_BIG + b[k] * f + d[k]
        r = np.maximum(t, 0.2 * t) + p[k] * M_BIG
        worst = max(worst, float(r.max()))
    return worst < -95.0


# ------------------------------------------------------------- numpy checker


def _numpy_device_sim(fs, fd, ms, valid, coef, layout):
    """Bit-level-ish simulation of the device program (for layout debug)."""
    a, b, c, d, p, s8 = coef
    deg, order, node_of, nb, W, colbase, F, groups = layout
    outs = []
    for ci in range(N_CORES):
        zsum = np.zeros((P, HEADS, nb), dtype=np.float32)
        wsum = np.zeros((P, HEADS, nb), dtype=np.float32)
        for k in range(HEADS):
            t = a[k] * fs[ci] + d[k]
            t = b[k] * fd[ci] + t
            t = c[k] * ms[ci] + t
            t = np.maximum(0.2 * t, t)
            t = p[k] * ms[ci] + t
            z = np.exp(t).astype(np.float32)
            if valid is not None:
                z = z * valid[ci]
            w = z * fs[ci]
            for (b0, cnt, Wg, off) in groups:
                zz = z[:, off : off + cnt * Wg].reshape(P, cnt, Wg)
                ww = w[:, off : off + cnt * Wg].reshape(P, cnt, Wg)
                zsum[:, k, b0 : b0 + cnt] = zz.sum(axis=2)
                wsum[:, k, b0 : b0 + cnt] = ww.sum(axis=2)
        den = zsum + np.float32(EPS_DEN)
        prod = wsum / den
        outb = prod.sum(axis=1) * s8  # [P, nb]
        outs.append(outb)
    return outs


def _assemble(outs, layout):
    deg, order, node_of, nb, W, colbase, F, groups = layout
    npc = node_of.shape[1]
    full = np.zeros(N_NODES, dtype=np.float32)
    jj = np.arange(npc)
    for ci in range(N_CORES):
        vals = outs[ci][jj % P, jj // P]  # [npc]
        nodes = node_of[ci]
        m = nodes >= 0
        full[nodes[m]] = vals[m]
    return full


# ------------------------------------------------------------- bass program


def _build_bass(F, nb, groups, coef, use_valid):
    import concourse.bass as bass
    import concourse.tile as tile
    from concourse import mybir
    import bass_rust

    def _split_excess_waits(nc, max_waits=1):
        """walrus codegen caps sync-wait commands per instruction; move extra
        sem waits onto dedicated same-engine NoOps placed just before."""
        ctr = [0]
        for bb in nc.main_func.blocks:
            new = []
            for ins in bb.instructions:
                si = ins.sync_info
                if si is not None and si.on_wait and len(si.on_wait) > max_waits:
                    waits = list(si.on_wait)
                    si.on_wait = waits[:max_waits]
                    extras = waits[max_waits:]
                    for i in range(0, len(extras), max_waits):
                        ctr[0] += 1
                        nop = mybir.InstNoOp(name=f"waitsplit-{ctr[0]}", ins=[], outs=[])
                        nop.engine = ins.engine
                        nop.sync_info = bass_rust.SyncInfo(
                            on_wait=extras[i : i + max_waits], on_update=[]
                        )
                        nc.register_instruction(nop, overwrite=True)
                        new.append(nop)
                new.append(ins)
            bb.instructions = new

    a, b, c, d, p, s8 = coef
    f32 = mybir.dt.float32
    bf16 = mybir.dt.bfloat16
    Alu = mybir.AluOpType
    Act = mybir.ActivationFunctionType

    nc = bass.Bass("TRN2")
    fs_d = nc.dram_tensor("fs", [P, F], bf16, kind="ExternalInput")
    fd_d = nc.dram_tensor("fd", [P, F], bf16, kind="ExternalInput")
    ms_d = nc.dram_tensor("ms", [P, F], bf16, kind="ExternalInput")
    dg_d = nc.dram_tensor("dg", [P, 4 * HEADS * P], bf16, kind="ExternalInput")
    if use_valid:
        va_d = nc.dram_tensor("va", [P, F], bf16, kind="ExternalInput")
    out_d = nc.dram_tensor("out", [P, nb], f32, kind="ExternalOutput")

    # column chunks (PSUM free-dim limit 512)
    chunks = []
    off = 0
    while off < F:
        cw = min(512, F - off)
        chunks.append((off, cw))
        off += cw

    with tile.TileContext(nc) as tc:
        with tc.tile_pool(name="pool", bufs=1) as pool, tc.tile_pool(
            name="psum", bufs=4, space="PSUM"
        ) as psum:
            fs = pool.tile([P, F], bf16)
            fd = pool.tile([P, F], bf16)
            ms = pool.tile([P, F], bf16)
            dg = pool.tile([P, 4 * HEADS * P], bf16)
            nc.gpsimd.dma_start(out=fs[:], in_=fs_d[:])
            nc.gpsimd.dma_start(out=fd[:], in_=fd_d[:])
            nc.gpsimd.dma_start(out=ms[:], in_=ms_d[:])
            nc.gpsimd.dma_start(out=dg[:], in_=dg_d[:])
            if use_valid:
                va = pool.tile([P, F], bf16)
                nc.gpsimd.dma_start(out=va[:], in_=va_d[:])
            ones = pool.tile([P, F], bf16)
            nc.vector.memset(ones[:], 1.0)

            zsum = pool.tile([P, HEADS, nb], f32)
            wsum = pool.tile([P, HEADS, nb], f32)

            uniform_p = bool(np.all(p == p[0]))
            if uniform_p:
                pm_u = pool.tile([P, F], bf16)
                nc.scalar.mul(out=pm_u[:], in_=ms[:], mul=float(p[0]))

            import contextlib

            _hstack = contextlib.ExitStack()
            hpool = _hstack.enter_context(tc.tile_pool(name="hpool", bufs=2))

            planes = [fs, fd, ms, ones]
            for k in range(HEADS):
                t1 = hpool.tile([P, F], bf16, tag="t1")
                t2 = hpool.tile([P, F], bf16, tag="t2")
                r0 = hpool.tile([P, F], bf16, tag="r0")
                z = hpool.tile([P, F], bf16, tag="z")
                w = hpool.tile([P, F], bf16, tag="w")
                # t = a*fs + b*fd + c*ms + d  via diag-stationary matmuls (PE)
                for (off, cw) in chunks:
                    tp = psum.tile([P, 512], f32, tag="tpsum")
                    for cf in range(4):
                        nc.tensor.matmul(
                            tp[:, :cw],
                            lhsT=dg[:, (k * 4 + cf) * P : (k * 4 + cf + 1) * P],
                            rhs=planes[cf][:, off : off + cw],
                            start=(cf == 0),
                            stop=(cf == 3),
                        )
                    # evict to bf16 plane on ScalarE
                    nc.scalar.copy(out=t1[:, off : off + cw], in_=tp[:, :cw])
                if uniform_p:
                    pm = pm_u
                else:
                    pm = hpool.tile([P, F], bf16, tag="pm")
                    nc.scalar.mul(out=pm[:], in_=ms[:], mul=float(p[k]))
                # lrelu + penalty on DVE: r = max(0.2t, t) + pm
                nc.vector.tensor_scalar(
                    out=t2[:], in0=t1[:], scalar1=0.2, scalar2=None, op0=Alu.mult
                )
                nc.vector.tensor_tensor(out=r0[:], in0=t2[:], in1=t1[:], op=Alu.max)
                nc.vector.tensor_tensor(out=r0[:], in0=r0[:], in1=pm[:], op=Alu.add)
                nc.scalar.activation(out=z[:], in_=r0[:], func=Act.Exp)
                if use_valid:
                    nc.vector.tensor_mul(out=z[:], in0=z[:], in1=va[:])
                nc.vector.tensor_mul(out=w[:], in0=z[:], in1=fs[:])
                for (b0, cnt, Wg, off) in groups:
                    zin = z[:, off : off + cnt * Wg].rearrange(
                        "p (c w) -> p c w", w=Wg
                    )
                    win = w[:, off : off + cnt * Wg].rearrange(
                        "p (c w) -> p c w", w=Wg
                    )
                    nc.vector.tensor_reduce(
                        out=zsum[:, k, b0 : b0 + cnt], in_=zin,
                        axis=mybir.AxisListType.X, op=Alu.add,
                    )
                    nc.vector.tensor_reduce(
                        out=wsum[:, k, b0 : b0 + cnt], in_=win,
                        axis=mybir.AxisListType.X, op=Alu.add,
                    )

            _hstack.close()

            # out[n] = s8 * sum_k wsum/(zsum+eps)
            den = pool.tile([P, HEADS, nb], f32)
            nc.vector.tensor_scalar(
                out=den[:], in0=zsum[:], scalar1=float(EPS_DEN), scalar2=None,
                op0=Alu.add,
            )
            rec = pool.tile([P, HEADS, nb], f32)
            nc.vector.reciprocal(out=rec[:], in_=den[:])
            prod = pool.tile([P, HEADS, nb], f32)
            nc.vector.tensor_mul(out=prod[:], in0=wsum[:], in1=rec[:])
            outb = pool.tile([P, nb], f32)
            nc.vector.tensor_reduce(
                out=outb[:],
                in_=prod[:].rearrange("p h n -> p n h"),
                axis=mybir.AxisListType.X,
                op=Alu.add,
            )
            outs = pool.tile([P, nb], f32)
            nc.vector.tensor_scalar(
                out=outs[:], in0=outb[:], scalar1=float(s8), scalar2=None,
                op0=Alu.mult,
            )
            nc.gpsimd.dma_start(out=out_d[:], in_=outs[:])
    _split_excess_waits(nc)
    return nc


# -------------------------------------------------------------------- kernel

_trace_flag = {"trace": False, "last": None}


def kernel(
    node_features,
    cycle_mask,
    W_proj,
    b_proj,
    W_att,
    b_att,
    cycle_penalty,
    min_sum_scaler,
    edge_index,
    _numpy=False,
):
    node_features = np.asarray(node_features)
    cycle_mask = np.asarray(cycle_mask)
    edge_index = np.asarray(edge_index)
    src = edge_index[0].astype(np.int64)
    dst = edge_index[1].astype(np.int64)

    coef = _fold_weights(
        np.asarray(W_proj), np.asarray(b_proj), np.asarray(W_att),
        np.asarray(b_att), np.asarray(cycle_penalty), np.asarray(min_sum_scaler),
    )
    layout = _build_layout(dst)
    use_valid = not _check_pad_trick(coef, node_features)
    fs, fd, ms, valid = _build_planes(
        node_features, cycle_mask, src, dst, layout, use_valid
    )
    deg, order, node_of, nb, W, colbase, F, groups = layout

    if _numpy:
        outs = _numpy_device_sim(fs, fd, ms, valid, coef, layout)
        return _assemble(outs, layout)

    from concourse.bass_utils import run_bass_kernel_spmd

    nc = _build_bass(F, nb, groups, coef, use_valid)
    import ml_dtypes

    bf = ml_dtypes.bfloat16
    a, b, c, d, p, s8 = coef
    dg = np.zeros((P, 4 * HEADS * P), dtype=np.float32)
    idx = np.arange(P)
    for k in range(HEADS):
        for cf, v in enumerate((a[k], b[k], c[k], d[k])):
            dg[idx, (k * 4 + cf) * P + idx] = v
    dg = dg.astype(bf)
    in_maps = []
    for ci in range(N_CORES):
        m = {
            "fs": fs[ci].astype(bf),
            "fd": fd[ci].astype(bf),
            "ms": ms[ci].astype(bf),
            "dg": dg,
        }
        if use_valid:
            m["va"] = valid[ci].astype(bf)
        in_maps.append(m)
    res = run_bass_kernel_spmd(
        nc, in_maps, core_ids=list(range(N_CORES)), trace=_trace_flag["trace"]
    )
    _trace_flag["last"] = res
    outs = [res.results[ci]["out"] for ci in range(N_CORES)]
    return _assemble(outs, layout)
